# revision 14
# baseline (speedup 1.0000x reference)
"""DualEdgeConv (gnn message passing) Trainium2 Bass kernel, 8-core SPMD.

Math: out[:, :64]  = segmax_row1( mlp([x1[r1], x1[c1]-x1[r1]]) )
      out[:, 64:]  = segmax_row2( mlp([x1[r2], x2[c2]-x1[r2]]) )
  mlp(e) = relu(e @ W1 + b1) @ W2 + b2
  With A = W1[:64]-W1[64:], B = W1[64:]:
      pre = A^T x1[row] + B^T x[col] + b1   (feature-major on device)

Device strategy (per core, SPMD):
- Edges sharded by destination row: core c owns rows [c*S, (c+1)*S).
- x1/x2 live in SBUF as packed-2 bf16 tables (entry m = [x[2m]|x[2m+1]],
  256B); per-slot col features fetched with SBUF-source dma_gather
  (transpose mode), idx = col>>1 (int16-safe). Col parity selects which
  64-partition half holds the features, so edges are split into parity
  streams, making the half a compile-time slice (no masks/selects).
- Per (set, parity): nodes' edge lists are degree-bucketed into classes K,
  padded to K slots by repeating a real col (max-invariant); per-class
  group counts padded to a common value across the 8 cores.
- Row features are NOT gathered: host ships x1[slice] transposed in each
  stream's class order; mm1's row term reads it with a stride-0
  (per-group broadcast) rhs access pattern.
- Per chunk (<=512 slots): psum1 = B^T @ cols  (+)  A^T @ rows_bcast;
  ACT relu+b1 -> bf16 h; psum2 = W2^T h; DVE tensor_reduce(max) over each
  group's K slots -> staging; staged [64, n] blocks DMA to DRAM.
- Host merges group maxima per node, adds b2, zeros empty nodes.
"""

import numpy as np
from ml_dtypes import bfloat16


# ---------------------------------------------------------------- config --

class Cfg:
    def __init__(self, N=50000, E=800000, ncores=8):
        self.N = N
        self.E = E
        self.D = 64
        self.H = 128
        self.NCORES = ncores
        self.SLICE = N // ncores
        self.CHUNK = 512        # max slots per compute chunk (one psum bank)
        self.SPAN_SLOTS = 2048  # col-gather batching (SWDGE fixed-cost amortize)
        self.STAGE_COLS = 1024  # output staging tile columns
        self.CLASSES = (list(range(1, 17)) +
                        [18, 20, 23, 26, 29, 32, 36, 41, 46, 52, 58,
                         64, 80, 96, 112, 128])


def pad128(n):
    return ((n + 127) // 128) * 128


# ----------------------------------------------------------- host tables --

def pack2_table(cfg, x):
    """x [N,64] fp32 -> packed-2 bf16 table [128, RANKS*128].
    Entry m = [x[2m] | x[2m+1]] at partition m%128, free (m//128)*128:+128
    (dma_gather SBUF source, tpr=128, 256B free per rank)."""
    xb = np.asarray(x).astype(bfloat16)
    M = (cfg.N + 1) // 2
    Mpad = pad128(M)
    ent = np.zeros((Mpad, 128), dtype=bfloat16)
    ent[:M] = xb.reshape(M, 128)
    R = Mpad // 128
    return np.ascontiguousarray(
        ent.reshape(R, 128, 128).transpose(1, 0, 2).reshape(128, R * 128))


def wrap16_replicate(idxs):
    """idxs [n] int16 (n%16==0) -> [128, n//16]: 16-partition wrap
    replicated 8x (one copy per Q7 core)."""
    n = idxs.shape[0]
    blk = idxs.reshape(n // 16, 16).T
    return np.ascontiguousarray(np.tile(blk, (8, 1)))


# ------------------------------------------------------------ host plan ---

def degree_class(cfg, d):
    for k in cfg.CLASSES:
        if d <= k:
            return k
    return cfg.CLASSES[-1]


def build_groups(cfg, rows, cols, core):
    lo, hi = core * cfg.SLICE, (core + 1) * cfg.SLICE
    sel = (rows >= lo) & (rows < hi)
    r = (rows[sel] - lo).astype(np.int64)
    c = cols[sel].astype(np.int64)
    out = {}
    for pi in (0, 1):
        m = (c & 1) == pi
        rp, cp = r[m], c[m]
        order = np.argsort(rp, kind="stable")
        rp, cp = rp[order], cp[order]
        uniq, start = np.unique(rp, return_index=True)
        start = np.append(start, rp.shape[0])
        by_class = {}
        for i in range(uniq.shape[0]):
            cs = cp[start[i]:start[i + 1]]
            d = cs.shape[0]
            o = 0
            while d - o > 0:
                take = min(d - o, cfg.CLASSES[-1])
                K = degree_class(cfg, take)
                grp = np.full(K, cs[o], dtype=np.int64)
                grp[:take] = cs[o:o + take]
                by_class.setdefault(K, []).append((int(uniq[i]), grp))
                o += take
        out[pi] = by_class
    return out


def build_plan(cfg, meta):
    """Flat device schedule, identical across cores.

    Per (s,pi): class segments {K, gc, n_chunks, spans}; each span is one
    col-gather: {n_chunks, col_off (idx elements), col_n (padded idx count),
    grp_off (group index at span start)}."""
    plans = []
    for (s, pi), cls in meta:
        segs = []
        col_pos = 0
        grp_pos = 0
        for K, n_chunks in cls:
            gc = cfg.CHUNK // K
            cs = gc * K
            per_span = max(1, cfg.SPAN_SLOTS // cs)
            spans = []
            ci = 0
            while ci < n_chunks:
                nc_ = min(per_span, n_chunks - ci)
                slots = nc_ * cs
                col_n = pad128(slots)
                spans.append(dict(n_chunks=nc_, col_off=col_pos, col_n=col_n,
                                  grp_off=grp_pos))
                col_pos += col_n
                grp_pos += nc_ * gc
                ci += nc_
            segs.append(dict(K=K, gc=gc, n_chunks=n_chunks, spans=spans))
        plans.append(dict(s=s, pi=pi, segs=segs, col_total=col_pos,
                          out_total=grp_pos))
    return plans


def preprocess(cfg, x1, x2, edge_index1, edge_index2, W1, b1, W2, b2):
    x1 = np.asarray(x1)
    tbl1 = pack2_table(cfg, x1)
    tbl2 = pack2_table(cfg, x2)

    sets = [np.asarray(edge_index1), np.asarray(edge_index2)]
    groups = [[build_groups(cfg, sets[s][0], sets[s][1], c) for s in (0, 1)]
              for c in range(cfg.NCORES)]

    meta = []
    for s in (0, 1):
        for pi in (0, 1):
            cls = []
            for K in cfg.CLASSES:
                gmax = max(len(groups[c][s][pi].get(K, []))
                           for c in range(cfg.NCORES))
                if gmax == 0:
                    continue
                gc = cfg.CHUNK // K
                G = ((gmax + gc - 1) // gc) * gc
                cls.append((K, G // gc))
            meta.append(((s, pi), cls))

    plans = build_plan(cfg, meta)

    per_core, decode = [], []
    for c in range(cfg.NCORES):
        xsl = x1[c * cfg.SLICE:(c + 1) * cfg.SLICE].astype(bfloat16)  # [S,64]
        colstreams, xrows, dec_core = [], [], []
        for plan in plans:
            s, pi = plan["s"], plan["pi"]
            col_idx = np.zeros(plan["col_total"], dtype=np.int16)
            nodes = np.zeros(plan["out_total"], dtype=np.int64)
            dec = np.full(plan["out_total"], -1, dtype=np.int64)
            for seg in plan["segs"]:
                K, gc = seg["K"], seg["gc"]
                glist = groups[c][s][pi].get(K, [])
                gi = 0
                for sp in seg["spans"]:
                    ngrp = sp["n_chunks"] * gc
                    cbuf = np.zeros((ngrp, K), dtype=np.int64)
                    for j in range(ngrp):
                        if gi < len(glist):
                            node, cols_g = glist[gi]
                            cbuf[j] = cols_g >> 1
                            nodes[sp["grp_off"] + j] = node
                            dec[sp["grp_off"] + j] = node
                        gi += 1
                    col_idx[sp["col_off"]:sp["col_off"] + ngrp * K] = \
                        cbuf.reshape(-1).astype(np.int16)
            colstreams.append(wrap16_replicate(col_idx))
            # class-ordered transposed row features [64, out_total]
            xrows.append(np.ascontiguousarray(xsl[nodes].T))
            dec_core.append(dec)
        col_off = []
        co = 0
        for a in colstreams:
            col_off.append(co)
            co += a.shape[1]
        inmap = {
            "tbl1": tbl1,
            "tbl2": tbl2,
            "colidx": np.concatenate(colstreams, axis=1),
        }
        # pack stream pairs into partition halves of one [128, n] array
        # (SBUF columns are a global resource across partitions)
        for half in range(len(xrows) // 2):
            a, b = xrows[2 * half], xrows[2 * half + 1]
            n = max(a.shape[1], b.shape[1])
            xp = np.zeros((128, n), dtype=bfloat16)
            xp[:64, :a.shape[1]] = a
            xp[64:, :b.shape[1]] = b
            inmap[f"xrow{half}"] = xp
        per_core.append((inmap, col_off))
        decode.append(dec_core)

    W1 = np.asarray(W1, dtype=np.float32)
    Dh = cfg.D
    wb = {
        "wA": np.ascontiguousarray(W1[:Dh] - W1[Dh:]).astype(bfloat16),
        "wB": np.ascontiguousarray(W1[Dh:]).astype(bfloat16),
        "wW2": np.asarray(W2, dtype=np.float32).astype(bfloat16),
        "b1": np.asarray(b1, dtype=np.float32).reshape(cfg.H, 1),
    }
    for inmap, _ in per_core:
        inmap.update(wb)

    return {
        "meta": meta, "plans": plans, "per_core": per_core,
        "decode": decode, "b2": np.asarray(b2, dtype=np.float32),
    }


def assemble(cfg, prep, acc_list):
    """acc_list[core][mi] = [64, out_total] fp32 group maxima. -> [N, 2D]"""
    b2 = prep["b2"]
    out = np.full((cfg.N, 2 * cfg.D), -np.inf, dtype=np.float32)
    for c in range(cfg.NCORES):
        lo = c * cfg.SLICE
        for mi, plan in enumerate(prep["plans"]):
            s = plan["s"]
            nodes = prep["decode"][c][mi]
            vals = acc_list[c][mi]
            valid = nodes >= 0
            np.maximum.at(out[:, s * cfg.D:(s + 1) * cfg.D],
                          lo + nodes[valid], vals.T[valid])
    empty = np.isinf(out)
    out[:, :cfg.D] += b2
    out[:, cfg.D:] += b2
    out[empty] = 0.0
    return out


# ---------------------------------------------------------- bass builder --

def build_bass(cfg, prep):
    import concourse.bacc as bacc
    import concourse.tile as tile
    from concourse import library_config, mybir

    plans = prep["plans"]
    inmap0 = prep["per_core"][0][0]
    col_offs = prep["per_core"][0][1]

    nc = bacc.Bacc(None, target_bir_lowering=False, debug=False)
    bf16 = mybir.dt.bfloat16
    f32 = mybir.dt.float32
    i16 = mybir.dt.int16

    din = {}
    for name, arr in inmap0.items():
        dt = {np.dtype(np.float32): f32, np.dtype(np.int16): i16,
              np.dtype(bfloat16): bf16}[arr.dtype]
        din[name] = nc.dram_tensor(name, list(arr.shape), dt,
                                   kind="ExternalInput")
    douts = []
    for mi, plan in enumerate(plans):
        douts.append(nc.dram_tensor(f"out{mi}", [64, plan["out_total"]], f32,
                                    kind="ExternalOutput"))

    RELU = mybir.ActivationFunctionType.Relu
    MAX = mybir.AluOpType.max
    AXX = mybir.AxisListType.X

    nc.gpsimd.load_library(library_config.mlp)
    with tile.TileContext(nc) as tc:
        with (
            tc.tile_pool(name="tbl", bufs=1) as tblp,
            tc.tile_pool(name="wp", bufs=1) as wp,
            tc.tile_pool(name="colg", bufs=3) as colg,
            tc.tile_pool(name="hp", bufs=4) as hpool,
            tc.tile_pool(name="op", bufs=2) as opool,
            tc.tile_pool(name="ps1", bufs=4, space="PSUM") as ps1,
            tc.tile_pool(name="ps2", bufs=4, space="PSUM") as ps2,
        ):
            # resident tables, idx streams, row features, weights
            tbl1_t = tblp.tile(list(inmap0["tbl1"].shape), bf16, tag="tbl1")
            tbl2_t = tblp.tile(list(inmap0["tbl2"].shape), bf16, tag="tbl2")
            cidx_t = tblp.tile(list(inmap0["colidx"].shape), i16, tag="cidx")
            nc.sync.dma_start(out=tbl1_t[:, :], in_=din["tbl1"][:, :])
            nc.sync.dma_start(out=tbl2_t[:, :], in_=din["tbl2"][:, :])
            nc.sync.dma_start(out=cidx_t[:, :], in_=din["colidx"][:, :])
            xrow_t = []
            for half in range(len(plans) // 2):
                xr = tblp.tile(list(inmap0[f"xrow{half}"].shape), bf16,
                               tag=f"xrow{half}")
                nc.sync.dma_start(out=xr[:, :], in_=din[f"xrow{half}"][:, :])
                xrow_t.append(xr)
            # wA/wB duplicated in both partition halves: matmul requires
            # lhsT.base_partition() == rhs.base_partition(); odd-parity
            # gathers and odd-stream xrow halves live in partitions 64:128.
            wA_t = wp.tile([128, 128], bf16, tag="wA")
            wB_t = wp.tile([128, 128], bf16, tag="wB")
            wW2_t = wp.tile([128, 64], bf16, tag="wW2")
            b1_t = wp.tile([cfg.H, 1], f32, tag="b1")
            nc.sync.dma_start(out=wA_t[0:64, :], in_=din["wA"][:, :])
            nc.sync.dma_start(out=wA_t[64:128, :], in_=din["wA"][:, :])
            nc.sync.dma_start(out=wB_t[0:64, :], in_=din["wB"][:, :])
            nc.sync.dma_start(out=wB_t[64:128, :], in_=din["wB"][:, :])
            nc.sync.dma_start(out=wW2_t[:, :], in_=din["wW2"][:, :])
            nc.sync.dma_start(out=b1_t[:, :], in_=din["b1"][:, :])

            for mi, plan in enumerate(plans):
                pi = plan["pi"]
                tbl_t = tbl1_t if plan["s"] == 0 else tbl2_t
                cbase = col_offs[mi]
                # output staging
                stag = opool.tile([64, cfg.STAGE_COLS], f32, tag="stag")
                soff = 0
                dram_off = 0
                for seg in plan["segs"]:
                    K, gc = seg["K"], seg["gc"]
                    cs = gc * K
                    for sp in seg["spans"]:
                        nch = sp["n_chunks"]
                        coln = sp["col_n"]
                        col_t = colg.tile([128, 1, cfg.SPAN_SLOTS], bf16,
                                          tag="colg")
                        o = cbase + sp["col_off"] // 16
                        nc.gpsimd.dma_gather(
                            col_t[:, :, :coln], tbl_t[:, :],
                            cidx_t[:, o:o + coln // 16],
                            num_idxs=coln, num_idxs_reg=coln,
                            elem_size=128, transpose=True,
                            sbuf_tokens_per_rank=128,
                            sbuf_free_dim_per_rank=256)
                        xrh = mi % 2  # partition half holding this stream's xrow
                        for ch in range(nch):
                            goff = sp["grp_off"] + ch * gc
                            p1 = ps1.tile([128, cs], f32, tag="p1")
                            colap = col_t[pi * 64:(pi + 1) * 64, 0,
                                          ch * cs:(ch + 1) * cs]
                            nc.tensor.matmul(
                                out=p1[:, :],
                                lhsT=wB_t[pi * 64:(pi + 1) * 64, :],
                                rhs=colap, start=True, stop=False)
                            rowap = xrow_t[mi // 2][xrh * 64:(xrh + 1) * 64,
                                                    goff:goff + gc] \
                                .to_broadcast([64, gc, K])
                            nc.tensor.matmul(
                                out=p1[:, :],
                                lhsT=wA_t[xrh * 64:(xrh + 1) * 64, :],
                                rhs=rowap, start=False, stop=True)
                            h_t = hpool.tile([128, cs], bf16, tag="h")
                            nc.scalar.activation(h_t[:, :], p1[:, :], RELU,
                                                 bias=b1_t[:, 0:1])
                            p2 = ps2.tile([64, cs], f32, tag="p2")
                            nc.tensor.matmul(out=p2[:, :], lhsT=wW2_t[:, :],
                                             rhs=h_t[:, :], start=True,
                                             stop=True)
                            if soff + gc > cfg.STAGE_COLS:
                                nc.sync.dma_start(
                                    out=douts[mi][:, dram_off:dram_off + soff],
                                    in_=stag[:, :soff])
                                dram_off += soff
                                soff = 0
                                stag = opool.tile([64, cfg.STAGE_COLS], f32,
                                                  tag="stag")
                            nc.vector.tensor_reduce(
                                out=stag[:, soff:soff + gc],
                                in_=p2[:, :].rearrange("p (g k) -> p g k",
                                                       k=K),
                                axis=AXX, op=MAX)
                            soff += gc
                if soff:
                    nc.sync.dma_start(
                        out=douts[mi][:, dram_off:dram_off + soff],
                        in_=stag[:, :soff])
    return nc


# ------------------------------------------------------------- execution --

def _run(cfg, prep, trace=False, tmpdir=None):
    from concourse.bass_utils import run_bass_kernel_spmd
    nc = build_bass(cfg, prep)
    nc.finalize()
    in_maps = [dict(prep["per_core"][c][0]) for c in range(cfg.NCORES)]
    res = run_bass_kernel_spmd(nc, in_maps, list(range(cfg.NCORES)),
                               trace=trace, tmpdir=tmpdir)
    accs = []
    for c in range(cfg.NCORES):
        accs.append([np.asarray(res.results[c][f"out{mi}"])
                     for mi in range(len(prep["plans"]))])
    return accs, res


def kernel(x1, x2, edge_index1, edge_index2, W1, b1, W2, b2):
    cfg = Cfg()
    assert x1.shape == (cfg.N, cfg.D)
    prep = preprocess(cfg, x1, x2, edge_index1, edge_index2, W1, b1, W2, b2)
    accs, _ = _run(cfg, prep)
    return assemble(cfg, prep, accs)
